# revision 2
# baseline (speedup 1.0000x reference)
"""Coordinate multi-strip attention (pooling) kernel for 8 TRN2 NeuronCores.

Full inputs in, full outputs out. Data-parallel over batch B=32 -> 4
samples per core; all parameters replicated.

Algebraic folding done on host (all linear, exact up to fp reassociation):
  strip = mean_w(x)                      (raw sum; /64 folded into K)
  u     = (strip + dw3(strip) + dw7(strip)) / 3   (7-tap per-channel conv)
  u_bn  = (u - mean)*gamma/sqrt(var+eps) + beta   (affine per channel)
  y     = conv1_w @ concat(u_bn_h, u_bn_w)        (1x1 conv, contraction over C)
=>  y[m,l] = sum_{c,d} K[m,c,d] * strip_raw[c,l+d] + yb[m]
with K[m,c,d] = conv1_w[m,c] * wcomb[c,d] * bn_scale[c] / 64 and the bias
terms folded into the BN1 affine. The TensorEngine computes this as 7
shifted matmuls per channel-half accumulating in PSUM.
"""

import numpy as np

import concourse.bass as bass
import concourse.mybir as mybir
import concourse.tile as tile
from concourse import bacc
from concourse.bass_utils import run_bass_kernel_spmd

EPS = 1e-5
F32 = mybir.dt.float32
N_CORES = 8
B_LOCAL = 4  # 32 / 8
C = 256
MIP = 8
O = 256
H = 64
W = 64

# g-build engine assignment per (b, cb): 'v' = vector engine, 'g' = gpsimd.
# Final multiply runs on the other engine. Tuned for DVE/GPSIMD balance.
_GBUILD_ON_GPSIMD = {(1, 1), (3, 1)}

_CACHE = {}


def _build_program():
    """Build + compile the Bass/Tile program once."""
    from contextlib import ExitStack

    nc = bacc.Bacc(
        "TRN2",
        target_bir_lowering=False,
        debug=False,
        enable_asserts=True,
        num_devices=N_CORES,
    )

    x_d = nc.dram_tensor("x", [B_LOCAL, C, H, W], F32, kind="ExternalInput")
    kt_d = nc.dram_tensor("kt", [2, 2, 128, 56], F32, kind="ExternalInput")
    wgt_d = nc.dram_tensor("wgt", [2, 8, 256], F32, kind="ExternalInput")
    sb_d = nc.dram_tensor("sb", [8, 8], F32, kind="ExternalInput")
    out_d = nc.dram_tensor("out", [B_LOCAL, C, H, W], F32, kind="ExternalOutput")

    mult = mybir.AluOpType.mult
    Relu = mybir.ActivationFunctionType.Relu
    Identity = mybir.ActivationFunctionType.Identity
    Sigmoid = mybir.ActivationFunctionType.Sigmoid

    with tile.TileContext(nc) as tc, ExitStack() as ctx:
        const = ctx.enter_context(tc.tile_pool(name="const", bufs=1))
        xpool = ctx.enter_context(tc.tile_pool(name="xp", bufs=8))
        gpool = ctx.enter_context(tc.tile_pool(name="gp", bufs=4))
        strips = ctx.enter_context(tc.tile_pool(name="strips", bufs=1))
        vq = ctx.enter_context(tc.tile_pool(name="vq", bufs=4))
        apool = ctx.enter_context(tc.tile_pool(name="ap", bufs=8))
        psum_y = ctx.enter_context(tc.tile_pool(name="py", bufs=4, space="PSUM"))
        psum_g = ctx.enter_context(tc.tile_pool(name="pg", bufs=4, space="PSUM"))

        # Constants
        kt_t = {}
        for dd in range(2):
            for cb in range(2):
                t = const.tile([128, 56], F32, tag=f"kt{dd}{cb}")
                nc.sync.dma_start(out=t[:], in_=kt_d[dd, cb])
                kt_t[dd, cb] = t
        wgt_t = {}
        for dd in range(2):
            t = const.tile([8, 256], F32, tag=f"wgt{dd}")
            nc.sync.dma_start(out=t[:], in_=wgt_d[dd])
            wgt_t[dd] = t
        sb_t = const.tile([8, 8], F32, tag="sb")
        nc.sync.dma_start(out=sb_t[:], in_=sb_d[:])

        # Strip tensors: [128c, 4b, 70] with 3-wide zero pads on both ends
        strip_t = {}
        for dd in range(2):
            for cb in range(2):
                t = strips.tile([128, B_LOCAL, 70], F32, tag=f"st{dd}{cb}")
                nc.gpsimd.memset(t[:, :, 0:3], 0.0)
                nc.gpsimd.memset(t[:, :, 67:70], 0.0)
                strip_t[dd, cb] = t

        for b in range(B_LOCAL):
            X = {}
            for cb in range(2):
                t = xpool.tile([128, H, W], F32, tag="X")
                nc.sync.dma_start(out=t[:], in_=x_d[b, cb * 128:(cb + 1) * 128])
                X[cb] = t

            # Raw strip sums.  dir 0: sum over w -> [c, h];  dir 1: sum over h -> [c, w]
            for cb in range(2):
                nc.vector.reduce_sum(
                    out=strip_t[0, cb][:, b, 3:67],
                    in_=X[cb][:],
                    axis=mybir.AxisListType.X,
                )
                nc.vector.reduce_sum(
                    out=strip_t[1, cb][:, b, 3:67],
                    in_=X[cb][:].rearrange("p h w -> p w h"),
                    axis=mybir.AxisListType.X,
                )

            # y_pre[m, l] via 7 shifted matmuls x 2 channel halves, PSUM-accumulated
            yp = {}
            for dd in range(2):
                p = psum_y.tile([8, 64], F32, tag="yp")
                n_mm = 0
                for cb in range(2):
                    for di in range(7):
                        nc.tensor.matmul(
                            p[:],
                            lhsT=kt_t[dd, cb][:, di * 8:(di + 1) * 8],
                            rhs=strip_t[dd, cb][:, b, di:di + 64],
                            start=(n_mm == 0),
                            stop=(n_mm == 13),
                        )
                        n_mm += 1
                yp[dd] = p

            # BN1 + hswish:  z = s1*yp + b1_dir;  v = z * min(relu(z+3), 6)
            # (the final /6 is folded into the gate weights)
            q = vq.tile([8, 2, 64], F32, tag="q")
            v = vq.tile([8, 2, 64], F32, tag="v")
            for dd in range(2):
                nc.scalar.activation(
                    out=q[:, dd], in_=yp[dd][:], func=Relu,
                    scale=sb_t[:, 0:1], bias=sb_t[:, 3 + dd:4 + dd],
                )
                nc.scalar.activation(
                    out=v[:, dd], in_=yp[dd][:], func=Identity,
                    scale=sb_t[:, 0:1], bias=sb_t[:, 1 + dd:2 + dd],
                )
            nc.vector.tensor_scalar_min(q[:], q[:], 6.0)
            nc.vector.tensor_mul(v[:], v[:], q[:])

            # Gates: a = sigmoid(Wg/6 @ v), per direction and channel half
            a = {}
            for dd in range(2):
                for cb in range(2):
                    ga = psum_g.tile([128, 64], F32, tag="ga")
                    nc.tensor.matmul(
                        ga[:],
                        lhsT=wgt_t[dd][:, cb * 128:(cb + 1) * 128],
                        rhs=v[:, dd],
                        start=True,
                        stop=True,
                    )
                    at = apool.tile([128, 64], F32, tag="a")
                    nc.scalar.activation(out=at[:], in_=ga[:], func=Sigmoid)
                    a[dd, cb] = at

            # g = a_h (x) a_w  broadcast outer product, then X *= g in place
            for cb in range(2):
                g = gpool.tile([128, H, W], F32, tag="g")
                ah = a[0, cb][:].broadcast_to([128, H, W])  # [c, h, w*]
                aw_ap = a[1, cb][:]
                aw = bass.AP(
                    aw_ap.tensor, aw_ap.offset,
                    [list(aw_ap.ap[0]), [0, H], list(aw_ap.ap[1])],
                )  # [c, h*, w]
                if (b, cb) in _GBUILD_ON_GPSIMD:
                    nc.gpsimd.tensor_tensor(g[:], ah, aw, mult)
                    nc.vector.tensor_tensor(X[cb][:], X[cb][:], g[:], mult)
                else:
                    nc.vector.tensor_tensor(g[:], ah, aw, mult)
                    nc.gpsimd.tensor_tensor(X[cb][:], X[cb][:], g[:], mult)
                nc.sync.dma_start(
                    out=out_d[b, cb * 128:(cb + 1) * 128], in_=X[cb][:]
                )

    nc.compile()
    return nc


def _fold_strip_params(w3, w7, gamma, beta, mean, var):
    scale = gamma / np.sqrt(var + EPS)  # [C]
    wc = np.zeros((C, 7), np.float64)
    wc[:, 3] += 1.0
    wc[:, 2:5] += w3.astype(np.float64)
    wc[:, :] += w7.astype(np.float64)
    wc /= 3.0
    Wt = wc * scale[:, None].astype(np.float64) / 64.0  # [C, 7]
    bias_c = beta - mean * scale  # [C]
    return Wt, bias_c


def _pack_params(inp):
    conv1 = inp["conv1_w"].astype(np.float64)  # [8, 256]
    kt = np.zeros((2, 2, 128, 56), np.float32)
    sb = np.zeros((8, 8), np.float32)
    s1 = inp["bn1_gamma"] / np.sqrt(inp["bn1_var"] + EPS)  # [8]

    for dd, pre in enumerate(("sph", "spw")):
        Wt, bias_c = _fold_strip_params(
            inp[f"{pre}_w3"], inp[f"{pre}_w7"], inp[f"{pre}_gamma"],
            inp[f"{pre}_beta"], inp[f"{pre}_mean"], inp[f"{pre}_var"],
        )
        # K[m, c, d] = conv1[m, c] * Wt[c, d]
        K = conv1[:, :, None] * Wt[None, :, :]  # [8, 256, 7]
        for cb in range(2):
            # kt[dd, cb, c_local, d*8 + m] = K[m, cb*128 + c_local, d]
            blk = K[:, cb * 128:(cb + 1) * 128, :]  # [8, 128, 7]
            kt[dd, cb] = blk.transpose(1, 2, 0).reshape(128, 56).astype(np.float32)
        yb = conv1 @ bias_c  # [8]
        b1 = (yb - inp["bn1_mean"]) * s1 + inp["bn1_beta"]  # [8]
        sb[:, 1 + dd] = b1.astype(np.float32)
        sb[:, 3 + dd] = (b1 + 3.0).astype(np.float32)

    sb[:, 0] = s1.astype(np.float32)

    wgt = np.zeros((2, 8, 256), np.float32)
    wgt[0] = (inp["convh_w"].T / 6.0).astype(np.float32)  # [m, o]
    wgt[1] = (inp["convw_w"].T / 6.0).astype(np.float32)
    return kt, wgt, sb


def kernel(**inputs):
    if "nc" not in _CACHE:
        _CACHE["nc"] = _build_program()
    nc = _CACHE["nc"]

    x = np.ascontiguousarray(inputs["x"], dtype=np.float32)
    kt, wgt, sb = _pack_params(inputs)

    in_maps = []
    for i in range(N_CORES):
        in_maps.append({
            "x": x[i * B_LOCAL:(i + 1) * B_LOCAL],
            "kt": kt,
            "wgt": wgt,
            "sb": sb,
        })
    res = run_bass_kernel_spmd(nc, in_maps, list(range(N_CORES)))
    out = np.concatenate([res.results[i]["out"] for i in range(N_CORES)], axis=0)
    return out


# revision 3
# speedup vs baseline: 1.0041x; 1.0041x over previous
"""Coordinate multi-strip attention (pooling) kernel for 8 TRN2 NeuronCores.

Full inputs in, full outputs out. Data-parallel over batch B=32 -> 4
samples per core; all parameters replicated.

Algebraic folding done on host (all linear, exact up to fp reassociation):
  strip = mean_w(x)                      (raw sum; /64 folded into K)
  u     = (strip + dw3(strip) + dw7(strip)) / 3   (7-tap per-channel conv)
  u_bn  = (u - mean)*gamma/sqrt(var+eps) + beta   (affine per channel)
  y     = conv1_w @ concat(u_bn_h, u_bn_w)        (1x1 conv, contraction over C)
=>  y[m,l] = sum_{c,d} K[m,c,d] * strip_raw[c,l+d] + yb[m]
with K[m,c,d] = conv1_w[m,c] * wcomb[c,d] * bn_scale[c] / 64 and the bias
terms folded into the BN1 affine. The TensorEngine computes this as 7
shifted matmuls per channel-half accumulating in PSUM (batched over pairs
of samples).

Engine split for the big streaming passes (SBUF-port contention between
GPSIMD and 2-port DVE ops is real, so DVE keeps only 1-port reduces plus
a few gate builds):
  DVE   : 16 strip reductions + g-builds for ~5 tiles
  GPSIMD: all 8 final multiplies + remaining g-builds
"""

import numpy as np

import concourse.bass as bass
import concourse.mybir as mybir
import concourse.tile as tile
from concourse import bacc
from concourse.bass_utils import run_bass_kernel_spmd

EPS = 1e-5
F32 = mybir.dt.float32
N_CORES = 8
B_LOCAL = 4  # 32 / 8
C = 256
H = 64
W = 64

# Per (b, cb): which engine builds g = a_h (x) a_w.  Final x*g multiply is
# always GPSIMD.  'v' = vector engine, 'g' = gpsimd.
_G_ENGINE = {
    (0, 0): 'v', (0, 1): 'v',
    (1, 0): 'v', (1, 1): 'g',
    (2, 0): 'v', (2, 1): 'g',
    (3, 0): 'v', (3, 1): 'g',
}

_CACHE = {}


def _build_program():
    from contextlib import ExitStack

    nc = bacc.Bacc(
        "TRN2",
        target_bir_lowering=False,
        debug=False,
        enable_asserts=True,
        num_devices=N_CORES,
    )

    x_d = nc.dram_tensor("x", [B_LOCAL, C, H, W], F32, kind="ExternalInput")
    kt_d = nc.dram_tensor("kt", [2, 2, 128, 56], F32, kind="ExternalInput")
    wgt_d = nc.dram_tensor("wgt", [2, 8, 256], F32, kind="ExternalInput")
    sb_d = nc.dram_tensor("sb", [8, 8], F32, kind="ExternalInput")
    out_d = nc.dram_tensor("out", [B_LOCAL, C, H, W], F32, kind="ExternalOutput")

    mult = mybir.AluOpType.mult
    Relu = mybir.ActivationFunctionType.Relu
    Identity = mybir.ActivationFunctionType.Identity
    Sigmoid = mybir.ActivationFunctionType.Sigmoid

    with tile.TileContext(nc) as tc, ExitStack() as ctx:
        const = ctx.enter_context(tc.tile_pool(name="const", bufs=1))
        xpool = ctx.enter_context(tc.tile_pool(name="xp", bufs=8))
        gpool = ctx.enter_context(tc.tile_pool(name="gp", bufs=4))
        strips = ctx.enter_context(tc.tile_pool(name="strips", bufs=1))
        vq = ctx.enter_context(tc.tile_pool(name="vq", bufs=2))
        apool = ctx.enter_context(tc.tile_pool(name="ap", bufs=4))
        psum_y = ctx.enter_context(tc.tile_pool(name="py", bufs=4, space="PSUM"))
        psum_g = ctx.enter_context(tc.tile_pool(name="pg", bufs=4, space="PSUM"))

        # Constants
        kt_t = {}
        for dd in range(2):
            for cb in range(2):
                t = const.tile([128, 56], F32, tag=f"kt{dd}{cb}")
                nc.sync.dma_start(out=t[:], in_=kt_d[dd, cb])
                kt_t[dd, cb] = t
        wgt_t = {}
        for dd in range(2):
            t = const.tile([8, 256], F32, tag=f"wgt{dd}")
            nc.sync.dma_start(out=t[:], in_=wgt_d[dd])
            wgt_t[dd] = t
        sb_t = const.tile([8, 8], F32, tag="sb")
        nc.sync.dma_start(out=sb_t[:], in_=sb_d[:])

        # Strip tensors: [128c, 4b, 70] with 3-wide zero pads on both ends
        strip_t = {}
        for dd in range(2):
            for cb in range(2):
                t = strips.tile([128, B_LOCAL, 70], F32, tag=f"st{dd}{cb}")
                nc.gpsimd.memset(t[:, :, 0:3], 0.0)
                nc.gpsimd.memset(t[:, :, 67:70], 0.0)
                strip_t[dd, cb] = t

        X = {}
        A = {}
        for pair in range(2):
            bs = (2 * pair, 2 * pair + 1)
            for b in bs:
                for cb in range(2):
                    t = xpool.tile([128, H, W], F32, tag="X")
                    nc.sync.dma_start(
                        out=t[:], in_=x_d[b, cb * 128:(cb + 1) * 128]
                    )
                    X[b, cb] = t
                for cb in range(2):
                    nc.vector.reduce_sum(
                        out=strip_t[0, cb][:, b, 3:67],
                        in_=X[b, cb][:],
                        axis=mybir.AxisListType.X,
                    )
                    nc.vector.reduce_sum(
                        out=strip_t[1, cb][:, b, 3:67],
                        in_=X[b, cb][:].rearrange("p h w -> p w h"),
                        axis=mybir.AxisListType.X,
                    )

            # y_pre for the pair: 7 shifted matmuls x 2 channel halves,
            # batched over the 2 samples (N=128), PSUM-accumulated.
            yp = {}
            for dd in range(2):
                p = psum_y.tile([8, 2, 64], F32, tag="yp")
                n_mm = 0
                for cb in range(2):
                    for di in range(7):
                        nc.tensor.matmul(
                            p[:],
                            lhsT=kt_t[dd, cb][:, di * 8:(di + 1) * 8],
                            rhs=strip_t[dd, cb][:, 2 * pair:2 * pair + 2,
                                                di:di + 64],
                            start=(n_mm == 0),
                            stop=(n_mm == 13),
                        )
                        n_mm += 1
                yp[dd] = p

            # BN1 + hswish:  z = s1*yp + b1_dir;  v = z * min(relu(z+3), 6)
            q = vq.tile([8, 2, 2, 64], F32, tag="q")  # [m, b2, dir, 64]
            v = vq.tile([8, 2, 2, 64], F32, tag="v")
            for dd in range(2):
                nc.scalar.activation(
                    out=q[:, :, dd], in_=yp[dd][:], func=Relu,
                    scale=sb_t[:, 0:1], bias=sb_t[:, 3 + dd:4 + dd],
                )
                nc.scalar.activation(
                    out=v[:, :, dd], in_=yp[dd][:], func=Identity,
                    scale=sb_t[:, 0:1], bias=sb_t[:, 1 + dd:2 + dd],
                )
            nc.vector.tensor_scalar_min(q[:], q[:], 6.0)
            nc.vector.tensor_mul(v[:], v[:], q[:])

            # Gates: a = sigmoid(Wg/6 @ v), batched over the pair (N=128)
            for dd in range(2):
                for cb in range(2):
                    ga = psum_g.tile([128, 2, 64], F32, tag="ga")
                    nc.tensor.matmul(
                        ga[:],
                        lhsT=wgt_t[dd][:, cb * 128:(cb + 1) * 128],
                        rhs=v[:, :, dd],
                        start=True,
                        stop=True,
                    )
                    at = apool.tile([128, 2, 64], F32, tag="a")
                    nc.scalar.activation(out=at[:], in_=ga[:], func=Sigmoid)
                    A[pair, dd, cb] = at

            # g = a_h (x) a_w ; X *= g ; store
            for b in bs:
                ip = b - 2 * pair
                for cb in range(2):
                    g = gpool.tile([128, H, W], F32, tag="g")
                    ah_ap = A[pair, 0, cb][:, ip]  # [128, 64]
                    aw_ap = A[pair, 1, cb][:, ip]
                    ah = ah_ap.broadcast_to([128, H, W])  # [c, h, w*]
                    aw = bass.AP(
                        aw_ap.tensor, aw_ap.offset,
                        [list(aw_ap.ap[0]), [0, H], list(aw_ap.ap[1])],
                    )  # [c, h*, w]
                    eng = nc.vector if _G_ENGINE[b, cb] == 'v' else nc.gpsimd
                    eng.tensor_tensor(g[:], ah, aw, mult)
                    nc.gpsimd.tensor_tensor(X[b, cb][:], X[b, cb][:], g[:], mult)
                    nc.sync.dma_start(
                        out=out_d[b, cb * 128:(cb + 1) * 128], in_=X[b, cb][:]
                    )

    nc.compile()
    return nc


def _fold_strip_params(w3, w7, gamma, beta, mean, var):
    scale = gamma / np.sqrt(var + EPS)  # [C]
    wc = np.zeros((C, 7), np.float64)
    wc[:, 3] += 1.0
    wc[:, 2:5] += w3.astype(np.float64)
    wc[:, :] += w7.astype(np.float64)
    wc /= 3.0
    Wt = wc * scale[:, None].astype(np.float64) / 64.0  # [C, 7]
    bias_c = beta - mean * scale  # [C]
    return Wt, bias_c


def _pack_params(inp):
    conv1 = inp["conv1_w"].astype(np.float64)  # [8, 256]
    kt = np.zeros((2, 2, 128, 56), np.float32)
    sb = np.zeros((8, 8), np.float32)
    s1 = inp["bn1_gamma"] / np.sqrt(inp["bn1_var"] + EPS)  # [8]

    for dd, pre in enumerate(("sph", "spw")):
        Wt, bias_c = _fold_strip_params(
            inp[f"{pre}_w3"], inp[f"{pre}_w7"], inp[f"{pre}_gamma"],
            inp[f"{pre}_beta"], inp[f"{pre}_mean"], inp[f"{pre}_var"],
        )
        K = conv1[:, :, None] * Wt[None, :, :]  # [8, 256, 7]
        for cb in range(2):
            blk = K[:, cb * 128:(cb + 1) * 128, :]  # [8, 128, 7]
            kt[dd, cb] = blk.transpose(1, 2, 0).reshape(128, 56).astype(np.float32)
        yb = conv1 @ bias_c  # [8]
        b1 = (yb - inp["bn1_mean"]) * s1 + inp["bn1_beta"]  # [8]
        sb[:, 1 + dd] = b1.astype(np.float32)
        sb[:, 3 + dd] = (b1 + 3.0).astype(np.float32)

    sb[:, 0] = s1.astype(np.float32)

    wgt = np.zeros((2, 8, 256), np.float32)
    wgt[0] = (inp["convh_w"].T / 6.0).astype(np.float32)  # [m, o]
    wgt[1] = (inp["convw_w"].T / 6.0).astype(np.float32)
    return kt, wgt, sb


def kernel(**inputs):
    if "nc" not in _CACHE:
        _CACHE["nc"] = _build_program()
    nc = _CACHE["nc"]

    x = np.ascontiguousarray(inputs["x"], dtype=np.float32)
    kt, wgt, sb = _pack_params(inputs)

    in_maps = []
    for i in range(N_CORES):
        in_maps.append({
            "x": x[i * B_LOCAL:(i + 1) * B_LOCAL],
            "kt": kt,
            "wgt": wgt,
            "sb": sb,
        })
    res = run_bass_kernel_spmd(nc, in_maps, list(range(N_CORES)))
    out = np.concatenate([res.results[i]["out"] for i in range(N_CORES)], axis=0)
    return out


# revision 5
# speedup vs baseline: 1.1451x; 1.1404x over previous
"""Coordinate multi-strip attention (pooling) kernel for 8 TRN2 NeuronCores.

Full inputs in, full outputs out. Data-parallel over batch B=32 -> 4
samples per core; all parameters replicated.

Algebraic folding done on host (all linear, exact up to fp reassociation):
  strip = mean_w(x)                      (raw sum; /64 folded into K)
  u     = (strip + dw3(strip) + dw7(strip)) / 3   (7-tap per-channel conv)
  u_bn  = (u - mean)*gamma/sqrt(var+eps) + beta   (affine per channel)
  y     = conv1_w @ concat(u_bn_h, u_bn_w)        (1x1 conv, contraction over C)
=>  y[m,l] = sum_{c,d} K[m,c,d] * strip_raw[c,l+d] + yb[m]
with K[m,c,d] = conv1_w[m,c] * wcomb[c,d] * bn_scale[c] / 64 and the bias
terms folded into the BN1 affine. The TensorEngine computes this as 7
shifted matmuls per channel-half accumulating in PSUM.

Engine assignment for the big streaming passes (GPSIMD contends with
2-port DVE ops for the shared SBUF port, so they are kept apart):
  ScalarE: transposed copy of each x tile so both strip reductions read
           dense (a strided DVE reduce costs 7.0us vs 4.4us dense)
  DVE    : 16 dense strip reductions + most g-builds
  GPSIMD : final x*g multiplies
  hswish smalls run through PSUM (no SBUF-port contention)
"""

import numpy as np

import concourse.bass as bass
import concourse.mybir as mybir
import concourse.tile as tile
from concourse import bacc
from concourse.bass_utils import run_bass_kernel_spmd

EPS = 1e-5
F32 = mybir.dt.float32
N_CORES = 8
B_LOCAL = 4  # 32 / 8
C = 256
H = 64
W = 64

# Per (b, cb): engine for the g-build / final multiply: 'v' = DVE, 'g' = gpsimd
_GATE_PLAN = {
    (0, 0): ('v', 'g'), (0, 1): ('v', 'g'),
    (1, 0): ('v', 'g'), (1, 1): ('v', 'g'),
    (2, 0): ('v', 'g'), (2, 1): ('v', 'g'),
    (3, 0): ('v', 'g'), (3, 1): ('g', 'v'),
}

_CACHE = {}


def _build_program():
    from contextlib import ExitStack

    nc = bacc.Bacc(
        "TRN2",
        target_bir_lowering=False,
        debug=False,
        enable_asserts=True,
        num_devices=N_CORES,
    )

    x_d = nc.dram_tensor("x", [B_LOCAL, C, H, W], F32, kind="ExternalInput")
    kt_d = nc.dram_tensor("kt", [2, 2, 128, 56], F32, kind="ExternalInput")
    wgt_d = nc.dram_tensor("wgt", [2, 8, 256], F32, kind="ExternalInput")
    sb_d = nc.dram_tensor("sb", [8, 8], F32, kind="ExternalInput")
    out_d = nc.dram_tensor("out", [B_LOCAL, C, H, W], F32, kind="ExternalOutput")

    mult = mybir.AluOpType.mult
    Relu = mybir.ActivationFunctionType.Relu
    Identity = mybir.ActivationFunctionType.Identity
    Sigmoid = mybir.ActivationFunctionType.Sigmoid
    Copy = mybir.ActivationFunctionType.Copy

    with tile.TileContext(nc) as tc, ExitStack() as ctx:
        const = ctx.enter_context(tc.tile_pool(name="const", bufs=1))
        xpool = ctx.enter_context(tc.tile_pool(name="xp", bufs=8))
        xtpool = ctx.enter_context(tc.tile_pool(name="xt", bufs=2))
        gpool = ctx.enter_context(tc.tile_pool(name="gp", bufs=2))
        strips = ctx.enter_context(tc.tile_pool(name="strips", bufs=1))
        vpool = ctx.enter_context(tc.tile_pool(name="vp", bufs=4))
        apool = ctx.enter_context(tc.tile_pool(name="ap", bufs=8))
        psum_y = ctx.enter_context(tc.tile_pool(name="py", bufs=4, space="PSUM"))
        psum_q = ctx.enter_context(tc.tile_pool(name="pq", bufs=2, space="PSUM"))
        psum_g = ctx.enter_context(tc.tile_pool(name="pg", bufs=2, space="PSUM"))

        # Constants
        kt_t = {}
        for dd in range(2):
            for cb in range(2):
                t = const.tile([128, 56], F32, tag=f"kt{dd}{cb}")
                nc.sync.dma_start(out=t[:], in_=kt_d[dd, cb])
                kt_t[dd, cb] = t
        wgt_t = {}
        for dd in range(2):
            t = const.tile([8, 256], F32, tag=f"wgt{dd}")
            nc.sync.dma_start(out=t[:], in_=wgt_d[dd])
            wgt_t[dd] = t
        sb_t = const.tile([8, 8], F32, tag="sb")
        nc.sync.dma_start(out=sb_t[:], in_=sb_d[:])

        # Strip tensors: [128c, 4b, 70] with 3-wide zero pads on both ends
        strip_t = {}
        for dd in range(2):
            for cb in range(2):
                t = strips.tile([128, B_LOCAL, 70], F32, tag=f"st{dd}{cb}")
                nc.gpsimd.memset(t[:, :, 0:3], 0.0)
                nc.gpsimd.memset(t[:, :, 67:70], 0.0)
                strip_t[dd, cb] = t

        X = {}
        for b in range(B_LOCAL):
            for cb in range(2):
                t = xpool.tile([128, H, W], F32, tag="X")
                nc.sync.dma_start(out=t[:], in_=x_d[b, cb * 128:(cb + 1) * 128])
                X[b, cb] = t

            for cb in range(2):
                # dense w-reduce straight off x
                nc.vector.reduce_sum(
                    out=strip_t[0, cb][:, b, 3:67],
                    in_=X[b, cb][:],
                    axis=mybir.AxisListType.X,
                )
                # ScalarE transposes the tile; h-reduce then reads dense
                xt = xtpool.tile([128, W, H], F32, tag="XT")
                nc.scalar.activation(
                    out=xt[:].rearrange("p w h -> p h w"), in_=X[b, cb][:],
                    func=Copy,
                )
                nc.vector.reduce_sum(
                    out=strip_t[1, cb][:, b, 3:67],
                    in_=xt[:],
                    axis=mybir.AxisListType.X,
                )

            # y_pre[m, l]: 7 shifted matmuls x 2 channel halves, PSUM-accumulated
            yp = {}
            for dd in range(2):
                p = psum_y.tile([8, 64], F32, tag="yp")
                n_mm = 0
                for cb in range(2):
                    for di in range(7):
                        nc.tensor.matmul(
                            p[:],
                            lhsT=kt_t[dd, cb][:, di * 8:(di + 1) * 8],
                            rhs=strip_t[dd, cb][:, b, di:di + 64],
                            start=(n_mm == 0),
                            stop=(n_mm == 13),
                        )
                        n_mm += 1
                yp[dd] = p

            # BN1 + hswish:  z = s1*yp + b1_dir;  v = z * min(relu(z+3), 6)
            # q lives in PSUM so the min/mul avoid the SBUF port entirely.
            q = psum_q.tile([8, 2, 64], F32, tag="q")
            v = vpool.tile([8, 2, 64], F32, tag="v")
            for dd in range(2):
                nc.scalar.activation(
                    out=q[:, dd], in_=yp[dd][:], func=Relu,
                    scale=sb_t[:, 0:1], bias=sb_t[:, 3 + dd:4 + dd],
                )
                nc.scalar.activation(
                    out=v[:, dd], in_=yp[dd][:], func=Identity,
                    scale=sb_t[:, 0:1], bias=sb_t[:, 1 + dd:2 + dd],
                )
            nc.vector.tensor_scalar_min(q[:], q[:], 6.0)
            nc.vector.tensor_mul(v[:], v[:], q[:])

            # Gates: a = sigmoid(Wg/6 @ v)
            a = {}
            for dd in range(2):
                for cb in range(2):
                    ga = psum_g.tile([128, 64], F32, tag="ga")
                    nc.tensor.matmul(
                        ga[:],
                        lhsT=wgt_t[dd][:, cb * 128:(cb + 1) * 128],
                        rhs=v[:, dd],
                        start=True,
                        stop=True,
                    )
                    at = apool.tile([128, 64], F32, tag="a")
                    nc.scalar.activation(out=at[:], in_=ga[:], func=Sigmoid)
                    a[dd, cb] = at

            # g = a_h (x) a_w ; X *= g ; store
            for cb in range(2):
                g_eng, m_eng = _GATE_PLAN[b, cb]
                g = gpool.tile([128, H, W], F32, tag="g")
                ah = a[0, cb][:].broadcast_to([128, H, W])  # [c, h, w*]
                aw_ap = a[1, cb][:]
                aw = bass.AP(
                    aw_ap.tensor, aw_ap.offset,
                    [list(aw_ap.ap[0]), [0, H], list(aw_ap.ap[1])],
                )  # [c, h*, w]
                eng1 = nc.vector if g_eng == 'v' else nc.gpsimd
                eng2 = nc.gpsimd if m_eng == 'g' else nc.vector
                eng1.tensor_tensor(g[:], ah, aw, mult)
                eng2.tensor_tensor(X[b, cb][:], X[b, cb][:], g[:], mult)
                nc.sync.dma_start(
                    out=out_d[b, cb * 128:(cb + 1) * 128], in_=X[b, cb][:]
                )

    nc.compile()
    return nc


def _fold_strip_params(w3, w7, gamma, beta, mean, var):
    scale = gamma / np.sqrt(var + EPS)  # [C]
    wc = np.zeros((C, 7), np.float64)
    wc[:, 3] += 1.0
    wc[:, 2:5] += w3.astype(np.float64)
    wc[:, :] += w7.astype(np.float64)
    wc /= 3.0
    Wt = wc * scale[:, None].astype(np.float64) / 64.0  # [C, 7]
    bias_c = beta - mean * scale  # [C]
    return Wt, bias_c


def _pack_params(inp):
    conv1 = inp["conv1_w"].astype(np.float64)  # [8, 256]
    kt = np.zeros((2, 2, 128, 56), np.float32)
    sb = np.zeros((8, 8), np.float32)
    s1 = inp["bn1_gamma"] / np.sqrt(inp["bn1_var"] + EPS)  # [8]

    for dd, pre in enumerate(("sph", "spw")):
        Wt, bias_c = _fold_strip_params(
            inp[f"{pre}_w3"], inp[f"{pre}_w7"], inp[f"{pre}_gamma"],
            inp[f"{pre}_beta"], inp[f"{pre}_mean"], inp[f"{pre}_var"],
        )
        K = conv1[:, :, None] * Wt[None, :, :]  # [8, 256, 7]
        for cb in range(2):
            blk = K[:, cb * 128:(cb + 1) * 128, :]  # [8, 128, 7]
            kt[dd, cb] = blk.transpose(1, 2, 0).reshape(128, 56).astype(np.float32)
        yb = conv1 @ bias_c  # [8]
        b1 = (yb - inp["bn1_mean"]) * s1 + inp["bn1_beta"]  # [8]
        sb[:, 1 + dd] = b1.astype(np.float32)
        sb[:, 3 + dd] = (b1 + 3.0).astype(np.float32)

    sb[:, 0] = s1.astype(np.float32)

    wgt = np.zeros((2, 8, 256), np.float32)
    wgt[0] = (inp["convh_w"].T / 6.0).astype(np.float32)  # [m, o]
    wgt[1] = (inp["convw_w"].T / 6.0).astype(np.float32)
    return kt, wgt, sb


def kernel(**inputs):
    if "nc" not in _CACHE:
        _CACHE["nc"] = _build_program()
    nc = _CACHE["nc"]

    x = np.ascontiguousarray(inputs["x"], dtype=np.float32)
    kt, wgt, sb = _pack_params(inputs)

    in_maps = []
    for i in range(N_CORES):
        in_maps.append({
            "x": x[i * B_LOCAL:(i + 1) * B_LOCAL],
            "kt": kt,
            "wgt": wgt,
            "sb": sb,
        })
    res = run_bass_kernel_spmd(nc, in_maps, list(range(N_CORES)))
    out = np.concatenate([res.results[i]["out"] for i in range(N_CORES)], axis=0)
    return out
